# revision 136
# baseline (speedup 1.0000x reference)
"""MultiHeadAttention (B=2, S=2048, D=1024, H=16, HD=64) on 8 TRN2 cores.

Sharding: core i -> batch b = i//4, head-group g = i%4 (4 heads = 256 channels).
Each core computes its 4 heads end-to-end (QKV projection slices, attention,
out-projection partials) and writes TWO [2048, 1024] fp32 partials (one per
head-PAIR, so the out-projection can start as soon as that pair's attention
block normalizes); host sums the 8 partials per batch and adds the constant
bias terms (WV_b@Wout_w + Wout_b, which commute through softmax averaging).

Schedule: ACT does 128 exps of [128,1024] (~1.11us each); PE streams
scores (quadrant-packed K=64 pairs) + attnV (M=65 with a ones-row producing
the softmax denominator for free) + QKV/out-proj fillers, ~1.25us/site.

Changes vs the 217us v2 (now ~212-215us measured):
 - TAIL rebuilt (~30us -> ~22 after the last exp; the last ~8us is the
   fixed end-of-kernel drain/barrier): ACT (idle after the final exp)
   computes 1/rowsum as Exp(-Ln(d)) straight off the PSUM rowsum rows
   (bass blocks the ACT Reciprocal table), a K=1 ones-matmul on the PE
   broadcasts it, the DVE multiplies SBUF*PSUM into the catT halves —
   replacing the mid-loop evict / reshape-DMA / DVE-recip / DMA-back /
   gpsimd-broadcast chain and ~4 of its ~2us DMA completion acks.  The
   spilled fillq o units + dummies keep HAM at 8/8 duty through the
   chain; evicts alternate vector/scalar and output DMAs alternate
   sync/scalar HWDGE (the Activation engine is the 2nd HWDGE queue).
 - qb0 de-PE-bound (~40us -> ~28 span): qb0 only reads vhx channels
   0:129, so V-proj runs as HALF-WIDTH units — pair 0 in qb0, pair 1
   (6.8us of PE) deferred to its own queue pulled 2/site at sites 60-75
   (deadline: attnV (1,0,kb) at site 64+kb).  qhT[1] sb2/sb3 (late2)
   similarly deferred to sites 80-87 (deadlines at sites 95/111).
 - fat DMA transfers: the 16 DMA engines move ~1 packet/~300ns
   REGARDLESS of size, and packet size = transfer bytes/128 — 0.25MB
   transfers stream ~110GB/s, 1MB ~420GB/s.  xq0/xk0 ride as single 1MB
   transfers, the xv stream as pairs/quads, deadline-interleaved with
   xk1-3; biases on gpsimd SWDGE (8-byte rows waste ~1us of HWDGE queue
   each).  fillq/pulls_qb0 re-derived for the new arrival order.

KEY STRUCTURAL FACTS (measured over ~35 builds):
 - every region sits within ~8% of an engine roofline: qb0 = PE filler
   work with in-qb0 deadlines; sites 16-63 = PE (late khT[1]/qhT[1]
   fillers, deadline site 63-75); steady = ACT exp (1113ns) + ~110ns;
   total is within ~8us of the startup+roofline+tail floor (~205us).
   Work moved out of one region bulges another — only DEADLINE slack
   (the vp1/late2 deferrals above) yields real wins.
 - wq0/wk0 via gpsimd SWDGE arrive ~14/19us (too late): SWDGE is only
   ~50GB/s-class for these; keep them on sync ahead of xq0/xk0.
 - some runs execute with the WHOLE CHIP at 5/6 clock (exp 1113->1337ns,
   ACT_TABLE_LOAD 1283->1539, total ~216us -> ~256us).  Late in a long
   session the device entered this state PERSISTENTLY (a build that had
   measured 217.0 re-measured 255.3) — it is device power/thermal state,
   not a kernel property.  If every engine looks uniformly ~20% slow,
   check ACT_TABLE_LOAD before blaming a code change.

Failed experiments (all measured SLOWER; don't retry blindly):
 - bulk input DMA on the scalar HWDGE queue: the tile scheduler HOISTS
   DMA instrs above the exps (no data dep), and one ring/sem-blocked
   instr stalls the whole in-order ACT queue — exp0 slipped 5-9us.
 - deadline round-robin across both HWDGE queues: the 8 GLOBAL
   completion-sem slots recycle with ~2us acks; slot order scrambles.
 - xv via gpsimd SWDGE: only ~50-96GB/s, and its packets starve the sync
   ramp exactly in the critical first ~15us, even delayed behind
   WAW-memset chains.
 - fewer/finer warm-up dummies, chunk-quads, or a K-proj slot between
   the Q-proj halves: any >~1us PE idle before ~30us re-gates HAM to 4/8
   for ~7-10us, running the projections 2x slow.
 - tail out-proj as split-K (K=64) pairs reading tmpL directly (to skip
   the catT cross-partition DMA): +4us — doubled PE drain time at tail
   clock beats the ~2us ack it saves.
Run-to-run variance of this config is ~±3us (HAM sensitivity).

Changes vs the 232us v1:
 - preamble: Q-side DMA first (Q-proj overlaps K-side DMA), weights split by
   head-pair, K-proj split kb01/kb23 so first scores fire earlier; a
   dummy-matmul burst warms the PE HAM clock gate before the projections
   (and again at the tail so the final out-proj matmuls run at 2.4GHz).
 - normalize: oAB still evicted to SBUF FIRST with plain copies (releasing
   the PSUM banks immediately — a late release stalls attnV in the in-order
   PE queue and lets HAM re-throttle); rowsum rows then reshape-DMA to
   [128,8] so the DVE reciprocal (8 cyc/elem, iterative-divide microcode)
   runs on 128 lanes (0.2us vs 4us in [2,512] row layout).
 - out-projection split per head-pair: single-matmul units, p0 partials
   drain mid-loop; only 8 matmuls + ~1MB of DMA remain after the last
   normalize. All output DMAs ride the sync HWDGE queue (empty once the
   input stream finishes ~28us in; the gpsimd SWDGE path measured ~4x
   slower and let output backlog leak into the tail).
 - output partials in bf16 (halves 16MB of output DMA; +~0.4% rms noise).

All matmul operands bf16 (fp8 rejected: attention output is cancellation-
suppressed to ~0.036 sigma_v, so elementwise fp8 noise passes through as
~4-7% relative output error vs the 2% gate). PSUM f32.
"""

import math

import numpy as np
import ml_dtypes

B, S, D, H = 2, 2048, 1024, 16
HD = 64
P = 128
NQ = S // 512  # 4 q-blocks of 512
NK = S // 128  # 16 k-blocks of 128
BF16 = ml_dtypes.bfloat16
N_DUMMY = 24  # PE warm-up matmuls (HAM clock-gate) before the projections

_CACHE = {}


def _build_nc():
    import concourse.bass as bass
    import concourse.mybir as mybir
    import concourse.tile as tile
    from concourse import bacc
    from concourse.bass import ds, ts

    f32 = mybir.dt.float32
    bf16 = mybir.dt.bfloat16

    nc = bacc.Bacc(None, target_bir_lowering=False, debug=False)

    xq_d = nc.dram_tensor("xq", [P, NQ, 8, 512], bf16, kind="ExternalInput")
    xk_d = nc.dram_tensor("xk", [P, NQ, 8, 512], bf16, kind="ExternalInput")
    xk0a_d = nc.dram_tensor("xk0a", [P, 8, 256], bf16, kind="ExternalInput")
    xk0b_d = nc.dram_tensor("xk0b", [P, 8, 256], bf16, kind="ExternalInput")
    xv_d = nc.dram_tensor("xv", [P, NK, 8, P], bf16, kind="ExternalInput")
    wq_d = nc.dram_tensor("wq", [P, 2, 8, P], bf16, kind="ExternalInput")
    wk_d = nc.dram_tensor("wk", [P, 2, 8, P], bf16, kind="ExternalInput")
    wfront_d = nc.dram_tensor("wfront", [P, 4096], bf16, kind="ExternalInput")
    wo_d = nc.dram_tensor("wo", [P, 2, 1024], bf16, kind="ExternalInput")
    bq_d = nc.dram_tensor("bq", [P, 2], f32, kind="ExternalInput")
    bk_d = nc.dram_tensor("bk", [P, 2], f32, kind="ExternalInput")
    # bf16 partials: halves 16MB of output DMA (the tail drain was ~17us);
    # 8 bf16 partials summed in f32 on host add ~0.4% rms vs the 2% gate
    out_d = nc.dram_tensor("out", [2, S, D], bf16, kind="ExternalOutput")

    with tile.TileContext(nc) as tc:
        with (
            tc.tile_pool(name="persist", bufs=1) as pp,
            tc.tile_pool(name="ps", bufs=2, space="PSUM") as ps_pool,
            tc.tile_pool(name="ps_out", bufs=2, space="PSUM") as po_pool,
            tc.tile_pool(name="fillps", bufs=2, space="PSUM") as fill_pool,
            tc.tile_pool(name="attn", bufs=4) as attn_pool,
            tc.tile_pool(name="small", bufs=4) as small_pool,
            tc.tile_pool(name="tail1", bufs=1) as tail1_pool,
        ):
            xq_sb = pp.tile([P, NQ, 8, 512], bf16, tag="xq_sb")
            xk_sb = pp.tile([P, NQ, 8, 512], bf16, tag="xk_sb")
            xv_sb = pp.tile([P, NK, 8, P], bf16, tag="xv_sb")
            # wq p0 + wk p0 + wv host-packed into one [P,4096] tensor so
            # the whole front weight set rides ONE 1MB 8KB-packet transfer
            # (~2.4us) instead of three 2-4KB-packet ones (~5.8us)
            wfront_sb = pp.tile([P, 4096], bf16, tag="wfront_sb")
            wq_sb = pp.tile([P, 1, 8, P], bf16, tag="wq_sb")
            wk_sb = pp.tile([P, 1, 8, P], bf16, tag="wk_sb")
            wo_sb = pp.tile([P, 2, 1024], bf16, tag="wo_sb")
            bq_sb = pp.tile([P, 2], f32, tag="bq_sb")
            bk_sb = pp.tile([P, 2], f32, tag="bk_sb")
            scr = pp.tile([P, 640], bf16, tag="scr")
            qhT = [
                pp.tile([P, S], bf16, tag=f"qhT{p}", name=f"qhT{p}")
                for p in range(2)
            ]
            khT = [
                pp.tile([P, S], bf16, tag=f"khT{p}", name=f"khT{p}")
                for p in range(2)
            ]
            vhx = pp.tile([P, NK, 260], bf16, tag="vhx")
            catT = [
                pp.tile([P, S], bf16, tag=f"catT{p}", name=f"catT{p}")
                for p in range(2)
            ]
            ones1 = pp.tile([1, 64], bf16, tag="ones1")
            nc.vector.memset(scr[:], 0.0)
            nc.gpsimd.memset(ones1[:], 1.0)
            for h in range(4):
                nc.vector.memset(vhx[:, :, 65 * h + 64], 1.0)

            # DMA schedule: same deadline interleave as the 217us version,
            # minus the K-side preamble (scalar HWDGE queue — the second
            # hardware DGE engine on trn2, otherwise idle until exp0) and
            # the biases (gpsimd SWDGE: tiny 8-byte-row transfers are slow
            # to issue and would hold a HWDGE completion-sem slot).  That
            # pulls every sync transfer ~4us earlier.  Scalar carries
            # nothing else before the exps: its DMA instrs get hoisted
            # ahead of the exps by the scheduler and a blocked one blocks
            # the whole ACT queue.
            # DMA schedule: the 217us baseline order, with the biases and
            # xv blocks 10-15 moved to the gpsimd SWDGE queue (separate
            # engine + sems, idle until the first o_units at ~site 58;
            # slow ~96GB/s but these have 20us+ of deadline slack).  That
            # drops ~1.7MB and 6 instrs from the sync stream, pulling xk /
            # xq_sb1 arrivals earlier through qb0.
            # DMA granularity: the 16 DMA engines process ~1 packet per
            # ~300ns REGARDLESS of packet size, and a transfer's packet
            # size is its per-partition row length (= bytes/128).  0.25MB
            # transfers (2KB packets) stream at ~110GB/s; 1MB transfers
            # (8KB packets) at ~420GB/s.  So: xq sb0, xk sb0 and the xv
            # stream ride as 1MB transfers; biases go to gpsimd (8-byte
            # rows would burn ~1us of queue time each for ~0 bytes).
            # DMA granularity: the 16 DMA engines move ~1 packet/~300ns
            # regardless of size, and packet size = transfer bytes/128.
            # 1MB transfers (8KB packets) stream ~420GB/s, 0.25MB (2KB)
            # only ~110.  xq0/xk0 ride as single 1MB transfers, the xv
            # stream as pairs/quads, deadline-interleaved with xk; the
            # 8-byte-row biases go to gpsimd (~1us of queue each for ~0
            # bytes).
            nc.gpsimd.dma_start(out=bq_sb[:], in_=bq_d[:])
            nc.gpsimd.dma_start(out=bk_sb[:], in_=bk_d[:])
            nc.sync.dma_start(out=wfront_sb[:], in_=wfront_d[:])
            nc.sync.dma_start(out=xq_sb[:, 0], in_=xq_d[:, 0])
            nc.sync.dma_start(out=xk_sb[:, 0], in_=xk_d[:, 0])
            nc.sync.dma_start(out=xv_sb[:, ds(0, 2)], in_=xv_d[:, ds(0, 2)])
            nc.sync.dma_start(out=xk_sb[:, 1], in_=xk_d[:, 1])
            nc.sync.dma_start(out=xv_sb[:, ds(2, 2)], in_=xv_d[:, ds(2, 2)])
            nc.sync.dma_start(out=xv_sb[:, ds(4, 4)], in_=xv_d[:, ds(4, 4)])
            nc.sync.dma_start(out=xk_sb[:, 2], in_=xk_d[:, 2])
            nc.sync.dma_start(out=xv_sb[:, ds(8, 4)], in_=xv_d[:, ds(8, 4)])
            nc.sync.dma_start(out=xk_sb[:, 3], in_=xk_d[:, 3])
            nc.sync.dma_start(out=xv_sb[:, ds(12, 4)], in_=xv_d[:, ds(12, 4)])
            nc.sync.dma_start(out=xq_sb[:, 1], in_=xq_d[:, 1])
            nc.sync.dma_start(out=wo_sb[:], in_=wo_d[:])
            nc.sync.dma_start(out=wk_sb[:, 0], in_=wk_d[:, 1])
            nc.sync.dma_start(out=wq_sb[:, 0], in_=wq_d[:, 1])
            nc.sync.dma_start(out=xq_sb[:, 2], in_=xq_d[:, 2])
            nc.sync.dma_start(out=xq_sb[:, 3], in_=xq_d[:, 3])

            def dummy():
                dm = fill_pool.tile([P, 512], f32, name="dummy_ps", tag="fill")
                nc.tensor.matmul(
                    dm[:], scr[:, 0:128], scr[:, ds(128, 512)],
                    start=True, stop=True,
                )

            # PE warm-up: the HAM clock gate needs ~3.4us of sustained PE
            # activity to unthrottle 1.2->2.4GHz; burn the DMA-wait on
            # dummy matmuls so the projections run warm.
            for _ in range(N_DUMMY):
                dummy()

            # ---- filler unit emitters (each unit ~2-4 matmuls of PE time)
            def qk_units(which, p, sb):
                x_sb, w1_sb, woff, b_sb, dst = (
                    (xq_sb, wq_sb, 0, bq_sb, qhT)
                    if which == "q"
                    else (xk_sb, wk_sb, 1024, bk_sb, khT)
                )
                state = {}

                def unit(c0, p=p, sb=sb):
                    if c0 == 0:
                        state["acc"] = fill_pool.tile(
                            [P, 512], f32, name="proj_ps", tag="fill"
                        )
                    acc = state["acc"]
                    for c in (c0, c0 + 1):
                        w_ap = (
                            wfront_sb[:, ds(woff + 128 * c, 128)]
                            if p == 0
                            else w1_sb[:, 0, c]
                        )
                        nc.tensor.matmul(
                            acc[:],
                            w_ap,
                            x_sb[:, sb, c],
                            start=(c == 0),
                            stop=(c == 7),
                        )
                    if c0 == 6:
                        nc.vector.tensor_scalar_add(
                            dst[p][:, ts(sb, 512)], acc[:], b_sb[:, ds(p, 1)]
                        )

                return [lambda c0=c0: unit(c0) for c0 in (0, 2, 4, 6)]

            def k0_half(lo):
                # K-proj p0 sb0, kb-pair granularity: columns [lo, lo+256)
                def unit(lo=lo):
                    acc = fill_pool.tile(
                        [P, 256], f32, name="kh_ps", tag="fill"
                    )
                    for c in range(8):
                        nc.tensor.matmul(
                            acc[:],
                            wfront_sb[:, ds(1024 + 128 * c, 128)],
                            xk_sb[:, 0, c, ds(lo, 256)],
                            start=(c == 0),
                            stop=(c == 7),
                        )
                    nc.vector.tensor_scalar_add(
                        khT[0][:, ds(lo, 256)], acc[:], bk_sb[:, ds(0, 1)]
                    )

                return unit

            def v_units(sb, vp=0):
                # half-width V projection: head-pair vp only.  qb0 needs
                # only pair 0 (vhx channels 0:129), so the pair-1 half
                # (6.8us of PE) is deferred out of the PE-bound qb0 phase
                # to sites 60-75 (deadline: attnV (1,0,kb) at site 64+kb)
                state = {}

                def unit(c0, sb=sb, vp=vp):
                    if c0 == 0:
                        state["acc"] = fill_pool.tile(
                            [P, 128], f32, name="vproj_ps", tag="fill"
                        )
                    acc = state["acc"]
                    for c in range(c0, c0 + 4):
                        nc.tensor.matmul(
                            acc[:],
                            xv_sb[:, sb, c],
                            wfront_sb[:, ds(2048 + 256 * c + 128 * vp, 128)],
                            start=(c == 0),
                            stop=(c == 7),
                        )
                    if c0 == 4:
                        for h in range(2):
                            nc.vector.tensor_copy(
                                out=vhx[:, sb, ds(65 * (2 * vp + h), 64)],
                                in_=acc[:, ds(64 * h, 64)],
                            )

                return [lambda: unit(0), lambda: unit(4)]

            def o_unit(sb, nh, p, fast_out=False, tail_evict=False, alt_q=False):
                def unit(sb=sb, nh=nh, p=p, fast_out=fast_out, tail_evict=tail_evict):
                    acc = fill_pool.tile(
                        [P, 512], f32, name="oproj_ps", tag="fill"
                    )
                    nc.tensor.matmul(
                        acc[:],
                        catT[p][:, ts(sb, P)],
                        wo_sb[:, p, ts(nh, 512)],
                        start=True,
                        stop=True,
                    )
                    osb = small_pool.tile([P, 512], bf16, name="oevict")
                    if tail_evict and (sb + nh) % 2:
                        # tail only: ACT is idle after the last exp; splitting
                        # the evicts across engines halves their serialization
                        nc.scalar.copy(out=osb[:], in_=acc[:])
                    else:
                        nc.vector.tensor_copy(out=osb[:], in_=acc[:])
                    # mid-loop outputs ride the gpsimd SWDGE so they never
                    # sit ahead of the normalize-chain DMAs in the sync FIFO
                    # (that delayed catT and stalled the PE at boundaries);
                    # the last blocks' outputs split across both HWDGE
                    # queues (scalar is free after the final exp)
                    eng = (
                        (nc.scalar if alt_q else nc.sync)
                        if fast_out
                        else nc.gpsimd
                    )
                    eng.dma_start(
                        out=out_d[p, ts(sb, P), ts(nh, 512)], in_=osb[:]
                    )

                return unit

            def inline(units):
                for u in units:
                    u()

            def emit_scores(p, qb, kb):
                sc = ps_pool.tile([P, 1024], f32, name="scores_ps", tag="ps")
                for ab in range(2):
                    nc.tensor.matmul(
                        sc[:, ds(512 * ab, 512)],
                        khT[p][ds(64 * ab, 64), ts(kb, P)],
                        qhT[p][ds(64 * ab, 64), ts(qb, 512)],
                        start=True,
                        stop=True,
                    )
                return sc

            def emit_normalize(p, qb, oAB, last=False):
                # Evict oAB to SBUF FIRST (plain copies): this releases the
                # PSUM banks immediately so the next blocks' attnV never
                # waits on the (latency-heavy) reciprocal chain — attnV sits
                # in the in-order PE queue, so a late release stalls the
                # whole PE and lets the HAM clock-gate re-throttle.
                o_sbs = []
                for ab in range(2):
                    o_sb = small_pool.tile([65, 512], f32, name="o_sb")
                    if last and ab == 1:
                        # tail only: ACT is idle after the final exp; run the
                        # second evict there so both copies go in parallel
                        nc.scalar.copy(out=o_sb[:], in_=oAB[ab][:])
                    else:
                        nc.vector.tensor_copy(out=o_sb[:], in_=oAB[ab][:])
                    o_sbs.append(o_sb)
                # rowsums live in row 64 (the vhx ones-column). Reshape-DMA
                # both rows into [128, 8] so the microcoded DVE reciprocal
                # (8 cyc/elem) runs on 128 lanes (0.2us vs 4us row-layout),
                # then DMA back to partition-0 rows for the gpsimd broadcast.
                # last block: split the chain's DMAs across both HWDGE
                # queues (ACT is free after the final exp) so the (1,2)
                # output drain on sync doesn't serialize ahead of them
                q0 = nc.scalar if last else nc.sync
                rT = small_pool.tile([P, 8], f32, name="rT")
                for ab in range(2):
                    (q0 if ab == 0 else nc.sync).dma_start(
                        out=rT[:, ds(4 * ab, 4)],
                        in_=o_sbs[ab][ds(64, 1), :],
                    )
                rrT = small_pool.tile([P, 8], f32, name="rrT")
                nc.vector.reciprocal(rrT[:], rT[:])
                rrow = [
                    small_pool.tile([1, 512], f32, name=f"rrow{ab}")
                    for ab in range(2)
                ]
                for ab in range(2):
                    (q0 if ab == 0 else nc.sync).dma_start(
                        out=rrow[ab][:], in_=rrT[:, ds(4 * ab, 4)]
                    )
                for ab in range(2):
                    bcs = small_pool.tile([64, 512], f32, name="bcast_sb")
                    nc.gpsimd.partition_broadcast(
                        bcs[:], rrow[ab][:], channels=64
                    )
                    if ab == 0:
                        nc.vector.tensor_tensor(
                            out=catT[p][ds(0, 64), ts(qb, 512)],
                            in0=o_sbs[0][ds(0, 64), :],
                            in1=bcs[:],
                            op=mybir.AluOpType.mult,
                        )
                    else:
                        tmp = small_pool.tile([64, 512], bf16, name="normB")
                        nc.vector.tensor_tensor(
                            out=tmp[:],
                            in0=o_sbs[1][ds(0, 64), :],
                            in1=bcs[:],
                            op=mybir.AluOpType.mult,
                        )
                        q0.dma_start(
                            out=catT[p][ds(64, 64), ts(qb, 512)], in_=tmp[:]
                        )

            # ---- pre-attention critical path: Q-proj sb0 (overlaps K-side
            # DMA), K-proj kb0-1, first scores, then vproj sb0.
            inline(qk_units("q", 0, 0))
            k0_half(0)()
            sc_next = emit_scores(0, 0, 0)
            inline(v_units(0))

            # ---- filler queue for qb0, arrival-ordered (xk0b ~17us,
            # xv0-2 ~18.5, xk1 ~20, xv3-5 ~21.5, xk2 ~23, xv6-8 ~25,
            # xk3 ~27, xv9-12 ~29, xq1 ~33, xv13-15 ~36)
            fillq = []
            fillq += [k0_half(256)]
            fillq += v_units(1) + v_units(2)
            fillq += qk_units("k", 0, 1)
            fillq += v_units(3) + v_units(4) + v_units(5) + v_units(6)
            fillq += qk_units("k", 0, 2)
            fillq += v_units(7) + v_units(8) + v_units(9) + v_units(10)
            fillq += qk_units("k", 0, 3)
            fillq += v_units(11) + v_units(12) + v_units(13) + v_units(14)
            fillq += qk_units("q", 0, 1)
            fillq += v_units(15)
            # per-site pull counts for qb0 (deadline-ordered fillq).
            # Cumulative pulls must cover each unit before its consumer is
            # EMITTED (in-order PE queue): k0b<=s1, v_sb<=s_sb,
            # ksbN<=s(4N-1), qsb1<=s15 (scores(0,1,0)).
            pulls_qb0 = [3, 2, 6, 2, 2, 2, 6, 2, 2, 2, 6, 2, 2, 2, 2, 6]
            # fillers for later sites (1 pull/site)
            late = []
            late += qk_units("q", 0, 2) + qk_units("q", 0, 3)
            for sb in range(NQ):
                late += qk_units("k", 1, sb)
            late += qk_units("q", 1, 0) + qk_units("q", 1, 1)
            # qhT[1] sb2/sb3 are needed only at scores(1,2,0)/(1,3,0)
            # (sites 95/111): deferred to sites 80-87, unloading the
            # PE-packed 16-55 region by ~3.4us of filler work
            late2 = qk_units("q", 1, 2) + qk_units("q", 1, 3)

            def pull(n):
                for _ in range(n):
                    if fillq:
                        u = fillq.pop(0)
                        if u is not None:
                            u()

            # deferred pair-1 V projection, pulled 2/site at sites 60-75
            # (its own queue: the main fillq's o units would FIFO ahead
            # of it past the vhx deadlines)
            vp1q = []
            for sb in range(NK):
                vp1q += v_units(sb, 1)

            def vpull(n):
                for _ in range(n):
                    if vp1q:
                        vp1q.pop(0)()

            # ---- attention site loop
            for p in range(2):
                for qb in range(NQ):
                    oAB = [
                        po_pool.tile(
                            [65, 512], f32, name=f"outT{ab}", tag="outT"
                        )
                        for ab in range(2)
                    ]
                    for kb in range(NK):
                        sc = sc_next
                        at = attn_pool.tile([P, 1024], bf16, name="attnT")
                        nc.scalar.activation(
                            at[:], sc[:], mybir.ActivationFunctionType.Exp
                        )
                        qb0 = (p, qb) == (0, 0)
                        if not qb0:
                            # ACT-bound phase: feed next scores immediately
                            if kb + 1 < NK:
                                sc_next = emit_scores(p, qb, kb + 1)
                            elif (p, qb) != (1, NQ - 1):
                                np_, nqb = (
                                    (p, qb + 1) if qb + 1 < NQ else (p + 1, 0)
                                )
                                sc_next = emit_scores(np_, nqb, 0)
                        if qb0:
                            # DMA-arrival-bound phase: pulls precede next
                            # scores so khT/qhT producers sit before their
                            # consumer in the in-order PE queue
                            pull(pulls_qb0[kb])
                            sc_next = emit_scores(
                                *((0, 0, kb + 1) if kb + 1 < NK else (0, 1, 0))
                            )
                        for ab in range(2):
                            nc.tensor.matmul(
                                oAB[ab][:],
                                vhx[:, kb, ds(65 * (2 * p + ab), 65)],
                                at[:, ds(512 * ab, 512)],
                                start=(kb == 0),
                                stop=(kb == NK - 1),
                            )
                        if not qb0:
                            s = p * 64 + qb * 16 + kb
                            if 60 <= s < 76:
                                vpull(2)
                            elif 80 <= s < 88 and late2:
                                late2.pop(0)()
                            else:
                                pull(1)
                    if (p, qb) == (0, 0):
                        fillq.extend(late)
                    if (p, qb) == (1, NQ - 1):
                        last_oAB = oAB
                    else:
                        emit_normalize(p, qb, oAB)
                    # out-projection partials for this head-pair can start
                    # now. APPEND (no interleave): the 'late' qk units ahead
                    # of them in fillq produce qhT[1]/khT[1], which MUST all
                    # be emitted before scores(1,0,0) at site ~63 — pulling
                    # them at 1/site from site 16 finishes by site 56.
                    if (p, qb) == (1, NQ - 1):
                        # last block's out-projection is emitted post-loop
                        continue
                    fasto = p == 1 and qb == NQ - 2
                    # two bubbles: o units consume catT written by the
                    # normalize chain (~6us incl DMA-completion hops); pulled
                    # too early they stall the in-order PE queue at each
                    # block boundary
                    fillq.append(None)
                    fillq.append(None)
                    for sb in range(4 * qb, 4 * qb + 4):
                        for nh in range(2):
                            fillq.append(o_unit(sb, nh, p, fast_out=fasto))
            # ---- tail: the last block's normalize, rebuilt for latency.
            # ACT (idle after the final exp) takes the reciprocal straight
            # off the PSUM rowsum rows; the PE broadcasts it with a K=1
            # ones matmul; the DVE multiplies into the catT halves reading
            # both operands from PSUM.  vs the mid-loop chain (evict +
            # reshape-DMA + recip + DMA-back + gpsimd broadcast) this
            # removes 4 DMA-completion hops (~2us ack each) from the
            # serial tail.
            for _ in range(3):
                dummy()
            # 1/d as Exp(-Ln(d)) — bass blocks the ACT Reciprocal table,
            # and the DVE reciprocal would need the [128,x] reshape hops
            # this path exists to avoid (rowsums are ~[500, 2e4], well
            # inside the Ln/Exp tables' accurate range)
            rrowL = []
            for ab in range(2):
                lnL = tail1_pool.tile([1, 512], f32, name="lnL")
                nc.scalar.activation(
                    lnL[:],
                    last_oAB[ab][ds(64, 1), :],
                    mybir.ActivationFunctionType.Ln,
                )
                rr = tail1_pool.tile([1, 512], bf16, name=f"rrowL{ab}")
                nc.scalar.activation(
                    rr[:],
                    lnL[:],
                    mybir.ActivationFunctionType.Exp,
                    scale=-1.0,
                )
                rrowL.append(rr)
            bcsL = []
            for ab in range(2):
                bc = ps_pool.tile([64, 512], f32, name="bcsL", tag="ps")
                nc.tensor.matmul(
                    bc[:], ones1[:], rrowL[ab][:], start=True, stop=True
                )
                bcsL.append(bc)
            # DVE can read only ONE operand from PSUM: evict the oAB data
            # rows to SBUF (rowsum row 64 stays in PSUM for ACT's Ln)
            o_sbL = []
            for ab in range(2):
                o_sb = tail1_pool.tile([64, 512], f32, name=f"o_sbL{ab}")
                nc.vector.tensor_copy(
                    out=o_sb[:], in_=last_oAB[ab][ds(0, 64), :]
                )
                o_sbL.append(o_sb)
            tmpL = tail1_pool.tile([64, 512], bf16, name="normBL")
            nc.vector.tensor_tensor(
                out=catT[1][ds(0, 64), ts(NQ - 1, 512)],
                in0=o_sbL[0][:],
                in1=bcsL[0][:],
                op=mybir.AluOpType.mult,
            )
            nc.vector.tensor_tensor(
                out=tmpL[:],
                in0=o_sbL[1][:],
                in1=bcsL[1][:],
                op=mybir.AluOpType.mult,
            )
            nc.scalar.dma_start(
                out=catT[1][ds(64, 64), ts(NQ - 1, 512)], in_=tmpL[:]
            )
            # the spilled fillq units (the last blocks' o units, ~14 after
            # the vp1 pull-pause) drain HERE — real PE work doubles as the
            # HAM duty-keeper while the catT DMA completes (the gate
            # dropped to 4/8 in the old tail, doubling the drain); dummies
            # top up the rest.  8 fine-grained o units (the small_pool
            # oevict ring is 4 deep): evicts alternate vector/scalar,
            # output DMAs alternate sync/scalar.
            while fillq:
                u = fillq.pop(0)
                if u is not None:
                    u()
            for _ in range(5):
                dummy()
            for sb in range(4 * (NQ - 1), 4 * NQ):
                for nh in range(2):
                    o_unit(
                        sb, nh, 1,
                        fast_out=True,
                        tail_evict=True,
                        alt_q=(sb + nh) % 2 == 0,
                    )()
                    dummy()

    nc.compile()
    return nc


def _prep_inputs(q, k, v, WQ_w, WQ_b, WK_w, WK_b, WV_w, WV_b, Wout_w, Wout_b):
    scale = 1.0 / math.sqrt(HD)

    def chunk_qk(x):  # [S, D] -> [P, NQ, 8, 512]
        return np.ascontiguousarray(
            x.T.reshape(8, P, NQ, 512).transpose(1, 2, 0, 3)
        ).astype(BF16)

    def chunk_v(x):  # [S, D] -> [P, NK, 8, 128]
        return np.ascontiguousarray(
            x.T.reshape(8, P, NK, P).transpose(1, 2, 0, 3)
        ).astype(BF16)

    xqs = [chunk_qk(q[b]) for b in range(B)]
    xks = [chunk_qk(k[b]) for b in range(B)]
    xvs = [chunk_v(v[b]) for b in range(B)]
    xk0as = [np.ascontiguousarray(x[:, 0, :, 0:256]) for x in xks]
    xk0bs = [np.ascontiguousarray(x[:, 0, :, 256:512]) for x in xks]

    in_maps = []
    for i in range(8):
        b, g = divmod(i, 4)
        sl = slice(256 * g, 256 * (g + 1))
        # [in_chunk(8), in_part(P), p(2), chan(P)] -> [in_part, p, in_chunk, chan]
        wq = (WQ_w[:, sl] * scale).reshape(8, P, 2, P).transpose(1, 2, 0, 3)
        wk = WK_w[:, sl].reshape(8, P, 2, P).transpose(1, 2, 0, 3)
        wv = WV_w[:, sl].reshape(8, P, 256).transpose(1, 0, 2)
        wfront = np.concatenate(
            [
                wq[:, 0].reshape(P, 1024),
                wk[:, 0].reshape(P, 1024),
                wv.reshape(P, 2048),
            ],
            axis=1,
        )
        wo = Wout_w[sl, :].reshape(2, P, 1024).transpose(1, 0, 2)
        bq = (WQ_b[sl] * scale).reshape(2, P).T
        bk = WK_b[sl].reshape(2, P).T
        in_maps.append(
            {
                "xq": xqs[b],
                "xk": xks[b],
                "xk0a": xk0as[b],
                "xk0b": xk0bs[b],
                "xv": xvs[b],
                "wq": np.ascontiguousarray(wq).astype(BF16),
                "wk": np.ascontiguousarray(wk).astype(BF16),
                "wfront": np.ascontiguousarray(wfront).astype(BF16),
                "wo": np.ascontiguousarray(wo).astype(BF16),
                "bq": np.ascontiguousarray(bq, dtype=np.float32),
                "bk": np.ascontiguousarray(bk, dtype=np.float32),
            }
        )
    return in_maps


def run(trace=False, **inputs):
    from concourse.bass_utils import run_bass_kernel_spmd

    if "nc" not in _CACHE:
        _CACHE["nc"] = _build_nc()
    nc = _CACHE["nc"]

    in_maps = _prep_inputs(**inputs)
    res = run_bass_kernel_spmd(nc, in_maps, list(range(8)), trace=trace)

    const = (
        inputs["WV_b"].astype(np.float32) @ inputs["Wout_w"].astype(np.float32)
        + inputs["Wout_b"].astype(np.float32)
    )
    out = np.zeros((B, S, D), dtype=np.float32)
    for i in range(8):
        b = i // 4
        r = res.results[i]["out"]
        out[b] += np.asarray(r[0], dtype=np.float32)
        out[b] += np.asarray(r[1], dtype=np.float32)
    out += const[None, None, :]
    return out, res


def kernel(**inputs):
    out, _ = run(trace=False, **inputs)
    return out

